# revision 5
# baseline (speedup 1.0000x reference)
"""GQA attention kernel for Trainium2, tensor-parallel over (batch, kv-head-pair).

Problem: B=2, S=2048, D=2048, 32 q heads / 8 kv heads, head_dim 64,
scores get an additive mask [1,1,S,S] + per-batch graph bias [B,1,S,S].

Sharding: 16 units = (batch 2) x (kv-head-pair 4) over 8 cores; core c handles
batch b = c % 2 and kv heads {2*(c//2), 2*(c//2)+1} (8 q heads). Each core
computes its heads' attention output and its slice of the wo matmul; the host
sums the 4 partial outputs per batch.

Fully fused single pipeline: projection blocks (A), attention pairs (B) and
output-projection groups (C) are interleaved in one emission stream so the PE
never drains between phases:
  - A(sc) projects seq block sc (512 cols) into xqT/xkT/xve; its x DMA is
    prefetched one block ahead.
  - B(g,t) = scoresT matmul -> exp (Act) -> ecomb multiply (DVE, bf16 2x
    mode) -> PV accumulate; the chain is pipelined with lookahead 2 and the
    ecomb tiles stream per-group (causal-trimmed).
  - After the last PV of group g the accumulator is copied to SBUF (so the
    single PSUM accumulator bank frees immediately) and normalized from the
    copy; C(g) (wo matmul + y DMA) is emitted a few pairs later so the PE
    queue never head-of-line blocks on the norm chain.
Matmul dtypes are uniform per phase (mixing 32/16-bit operands is illegal):
f32r avoids InstLdweights, bf16 halves DMA/SBUF traffic.
"""

import sys

if "/opt/trn_rl_repo" not in sys.path:
    sys.path.insert(0, "/opt/trn_rl_repo")

import numpy as np
import ml_dtypes
from contextlib import ExitStack

import concourse.bass as bass
import concourse.tile as tile
from concourse import bacc, mybir
from concourse.bass_utils import run_bass_kernel_spmd
from concourse.masks import make_identity

F32 = mybir.dt.float32
BF16 = mybir.dt.bfloat16
F32R = mybir.dt.float32r

D = 2048          # model dim
HD = 64           # head dim
NREP = 4          # q heads per kv head
NKVL = 2          # kv heads per core
N_CORES = 8
DOUT_Q = NREP * NKVL * HD   # 512
WCOLS = DOUT_Q + 2 * NKVL * HD  # 768

# per-phase matmul dtypes (both operands must match within one matmul)
A_DT = BF16   # projections: w stationary, x moving
B_DT = BF16   # scores (xkT stationary, xqT moving) + PV (xve stat, eT moving)
C_DT = BF16   # output projection: attnT stationary, wo moving

C_LAG = 6     # pairs between norm emission and C emission (hide norm latency)


def build_program(S=2048, causal=False, loop_n=1):
    G = S // 128   # q groups
    TK = S // 128  # kpos tiles
    NSC = S // 512  # A blocks
    assert S % 512 == 0

    nc = bacc.Bacc("TRN2", target_bir_lowering=False, debug=False,
                   num_devices=N_CORES)
    xdt = F32R if A_DT == F32R else BF16
    wodt = F32R if C_DT == F32R else BF16
    xT_d = nc.dram_tensor("xT", (D, S), xdt, kind="ExternalInput").ap()
    ecombT_d = nc.dram_tensor("ecombT", (S, S), BF16, kind="ExternalInput").ap()
    wqkv_d = nc.dram_tensor("wqkv", (D, WCOLS), xdt, kind="ExternalInput").ap()
    wo_d = nc.dram_tensor("wo", (DOUT_Q, D), wodt, kind="ExternalInput").ap()
    y_d = nc.dram_tensor("y", (S, D), BF16, kind="ExternalOutput").ap()

    with tile.TileContext(nc) as tc, ExitStack() as ctx:
        def body():
            with ExitStack() as bctx:
                persist = bctx.enter_context(tc.tile_pool(name="persist", bufs=1))
                xqT = persist.tile([128, NREP * S], B_DT)       # [(kvl,d),(rep,q)]
                xkT = persist.tile([128, S], B_DT)              # [(kvl,d), kpos]
                xve = persist.tile([128, NKVL * TK * 65], B_DT)  # [kpos,(kvl,tk,d+1)]
                attnT = persist.tile([128, NREP * S], C_DT)     # [(kvl,d),(rep,q)]
                w_sb = persist.tile([128, 16 * WCOLS], A_DT)
                wo_sb = persist.tile([128, NREP * D], C_DT)
                ident = persist.tile([128, 128], BF16)
                xqT3 = xqT.rearrange("p (h q) -> p h q", h=NREP)
                attnT3 = attnT.rearrange("p (h q) -> p h q", h=NREP)
                xve4 = xve.rearrange("p (v t c) -> p v t c", v=NKVL, c=65)
                w3 = w_sb.rearrange("p (t o) -> p t o", t=16)
                wo3 = wo_sb.rearrange("p (r n) -> p r n", r=NREP)
                if B_DT == F32R:
                    nc.vector.memset(xve4[:, :, :, 64:65].bitcast(F32), 1.0)
                else:
                    nc.vector.memset(xve4[:, :, :, 64:65], 1.0)
                make_identity(nc, ident)

                wsrc = wqkv_d.rearrange("(t p) o -> p t o", p=128)
                wosrc = wo_d.rearrange("(r p) n -> p r n", p=128)

                # streaming pools
                xsp = bctx.enter_context(tc.tile_pool(name="xs", bufs=2))
                ecp = bctx.enter_context(tc.tile_pool(name="ec", bufs=2))
                esp = bctx.enter_context(tc.tile_pool(name="eS", bufs=3))
                etp = bctx.enter_context(tc.tile_pool(name="eT", bufs=3))
                opc = bctx.enter_context(tc.tile_pool(name="opc", bufs=2))
                nrm = bctx.enter_context(tc.tile_pool(name="nrm", bufs=2))
                yp = bctx.enter_context(tc.tile_pool(name="y", bufs=2))
                vfp = bctx.enter_context(tc.tile_pool(name="vts", bufs=2))
                psS = bctx.enter_context(
                    tc.tile_pool(name="psS", bufs=2, space="PSUM"))
                psO = bctx.enter_context(
                    tc.tile_pool(name="psO", bufs=1, space="PSUM"))
                pmix = bctx.enter_context(
                    tc.tile_pool(name="pmix", bufs=2, space="PSUM"))

                # weight loads (once per body); wo later (first needed ~g=0 C)
                for wc in range(6):
                    nc.sync.dma_start(w3[:, :, wc * 128:(wc + 1) * 128],
                                      wsrc[:, :, wc * 128:(wc + 1) * 128])

                # ---------------- A blocks ----------------
                xblks = {}

                def emit_A_dma(sc):
                    xblk = xsp.tile([128, 16 * 512], A_DT, tag="xblk",
                                    name="xblk")
                    xb3 = xblk.rearrange("p (t s) -> p t s", t=16)
                    src = xT_d[:, sc * 512:(sc + 1) * 512] \
                        .rearrange("(t p) s -> p t s", p=128)
                    nc.sync.dma_start(xb3[:, 0:8, :], src[:, 0:8, :])
                    nc.sync.dma_start(xb3[:, 8:16, :], src[:, 8:16, :])
                    xblks[sc] = xb3

                def emit_A_mm(sc):
                    xb3 = xblks.pop(sc)
                    for j in range(6):
                        psJ = pmix.tile([128, 512], F32, tag="mix", name="psj")
                        for tin in range(16):
                            nc.tensor.matmul(
                                psJ,
                                w3[:, tin, j * 128:(j + 1) * 128],
                                xb3[:, tin, :],
                                start=(tin == 0), stop=(tin == 15))
                        if j < 4:
                            nc.scalar.copy(
                                xqT3[:, j, sc * 512:(sc + 1) * 512], psJ)
                        elif j == 4:
                            nc.scalar.copy(xkT[:, sc * 512:(sc + 1) * 512], psJ)
                        else:
                            vts = vfp.tile([128, 512], BF16, tag="vts",
                                           name="vts")
                            nc.vector.tensor_copy(vts, psJ)
                            pvr = pmix.tile([128, 512], F32, tag="mix",
                                            name="pvt")
                            pvt = pvr.bitcast(BF16)
                            for jj in range(4):
                                nc.tensor.transpose(
                                    pvt[:, jj * 128:(jj + 1) * 128],
                                    vts[:, jj * 128:(jj + 1) * 128], ident)
                            src = pvt[:, 0:512].rearrange(
                                "p (t v c) -> p v t c", t=4, v=NKVL)
                            nc.vector.tensor_copy(
                                xve4[:, :, sc * 4:(sc + 1) * 4, 0:64], src)

                # ---------------- B pipeline ----------------
                def emit_ec(g, tmax):
                    ec = ecp.tile([128, TK * 128], BF16, tag="ec", name="ec")
                    ec3 = ec.rearrange("p (t q) -> p t q", t=TK)
                    src = ecombT_d[0:tmax * 128, g * 128:(g + 1) * 128] \
                        .rearrange("(t p) q -> p t q", p=128)
                    nc.sync.dma_start(ec3[:, 0:tmax, :], src)
                    return ec3

                oP_box = [None]
                eTq = []           # pending (g, t, tmax, eT)
                pending_C = []     # (g, emit_after_pair_index)

                def emit_score(g, t, tmax, ec3):
                    sS = psS.tile([128, 1024], F32, tag="ps", name="ps")
                    for kvl in range(NKVL):
                        p0, p1 = kvl * 64, (kvl + 1) * 64
                        nc.tensor.matmul(
                            sS[:, kvl * 512:(kvl + 1) * 512],
                            xkT[p0:p1, t * 128:(t + 1) * 128],
                            xqT3[p0:p1, :, g * 128:(g + 1) * 128],
                            start=True, stop=True)
                    eS = esp.tile([128, 1024], BF16, tag="eS", name="eS")
                    nc.scalar.activation(eS, sS,
                                         mybir.ActivationFunctionType.Exp,
                                         scale=0.125)
                    eT = etp.tile([128, 1024], BF16 if B_DT == BF16 else F32R,
                                  tag="eT", name="eT")
                    in1 = (ec3[:, t:t + 1, :]
                           .unsqueeze(2).broadcast_to((128, 1, 2 * NREP, 128)))
                    nc.vector.tensor_mul(
                        eT.rearrange("p (o h q) -> p o h q", o=1, h=2 * NREP),
                        eS.rearrange("p (o h q) -> p o h q", o=1, h=2 * NREP),
                        in1)
                    eTq.append((g, t, tmax, eT))

                def emit_norm(g, oPc):
                    rec = nrm.tile([1, 1024], F32, tag="rec", name="rec")
                    nc.vector.reciprocal(rec, oPc[64:65, :])
                    recb = nrm.tile([64, 1024], F32, tag="recb", name="recb")
                    nc.gpsimd.partition_broadcast(recb, rec)
                    rec4 = recb.rearrange("p (v h q) -> p v h q", v=NKVL,
                                          h=NREP)
                    nc.gpsimd.tensor_mul(
                        attnT3[0:64, :, g * 128:(g + 1) * 128],
                        oPc[0:64, 0:512].rearrange("p (h q) -> p h q", h=NREP),
                        rec4[:, 0])
                    shift = nrm.tile([64, 512], C_DT, tag="shift", name="shift")
                    nc.gpsimd.tensor_mul(
                        shift.rearrange("p (h q) -> p h q", h=NREP),
                        oPc[0:64, 512:1024].rearrange("p (h q) -> p h q",
                                                      h=NREP),
                        rec4[:, 1])
                    nc.sync.dma_start(
                        attnT3[64:128, :, g * 128:(g + 1) * 128],
                        shift.rearrange("p (h q) -> p h q", h=NREP))

                def emit_C(g):
                    y_sb = yp.tile([128, D], BF16, tag="ysb", name="ysb")
                    for nch in range(4):
                        pY = pmix.tile([128, 512], F32, tag="mix", name="py")
                        for r in range(NREP):
                            nc.tensor.matmul(
                                pY,
                                attnT3[:, r, g * 128:(g + 1) * 128],
                                wo3[:, r, nch * 512:(nch + 1) * 512],
                                start=(r == 0), stop=(r == NREP - 1))
                        if nch % 2 == 0:
                            nc.vector.tensor_copy(
                                y_sb[:, nch * 512:(nch + 1) * 512], pY)
                        else:
                            nc.scalar.copy(
                                y_sb[:, nch * 512:(nch + 1) * 512], pY)
                    nc.sync.dma_start(y_d[g * 128:(g + 1) * 128, :], y_sb)

                def emit_pv(cur_i):
                    g, t, tmax, eT = eTq.pop(0)
                    if t == 0:
                        oP_box[0] = psO.tile([128, 1024], F32, tag="po",
                                             name="po")
                    oP = oP_box[0]
                    for kvl in range(NKVL):
                        nc.tensor.matmul(
                            oP[0:65, kvl * 512:(kvl + 1) * 512],
                            xve4[:, kvl, t, :],
                            eT[:, kvl * 512:(kvl + 1) * 512],
                            start=(t == 0), stop=(t == tmax - 1))
                    if t == tmax - 1:
                        # fast PSUM->SBUF copy so the single psO buffer frees
                        # for the next group's PV chain immediately.
                        oPc = opc.tile([128, 1024], F32, tag="opc", name="opc")
                        nc.scalar.copy(oPc, oP)
                        emit_norm(g, oPc)
                        pending_C.append((g, cur_i + C_LAG))

                # ---------------- schedule ----------------
                pairs = []
                for g in range(G):
                    tmax = min(g + 1, TK) if causal else TK
                    for t in range(tmax):
                        pairs.append((g, t, tmax))

                emit_A_dma(0)
                emit_A_mm(0)
                if NSC > 1:
                    emit_A_dma(1)
                nc.sync.dma_start(wo3[:, 0:2, :], wosrc[:, 0:2, :])
                nc.sync.dma_start(wo3[:, 2:4, :], wosrc[:, 2:4, :])

                done_sc = 1
                last_g = -1
                ec3_of = {}
                for i, (g, t, tmax) in enumerate(pairs):
                    if g != last_g:
                        need = max(g, tmax - 1) // 4
                        while done_sc <= need:
                            emit_A_mm(done_sc)
                            done_sc += 1
                            if done_sc < NSC:
                                emit_A_dma(done_sc)
                        ec3_of[g] = emit_ec(g, tmax)
                        ec3_of.pop(g - 2, None)
                        last_g = g
                    emit_score(g, t, tmax, ec3_of[g])
                    while pending_C and pending_C[0][1] <= i:
                        emit_C(pending_C.pop(0)[0])
                    if i >= 2:
                        emit_pv(i)
                while eTq:
                    emit_pv(len(pairs))
                while pending_C:
                    emit_C(pending_C.pop(0)[0])
                # non-causal / leftover A blocks (shouldn't happen, but safe)
                while done_sc < NSC:
                    emit_A_mm(done_sc)
                    done_sc += 1
                    if done_sc < NSC:
                        emit_A_dma(done_sc)

        for _rep in range(loop_n):
            body()

    nc.compile()
    return nc


def shard_inputs(x, mask, graph_bias, wq, wk, wv, wo, S=2048):
    """Build the 8 per-core input maps from the full inputs."""
    mask2 = np.asarray(mask, dtype=np.float32).reshape(S, S)
    gb = np.asarray(graph_bias, dtype=np.float32).reshape(2, S, S)
    ecombT_b = [np.ascontiguousarray(
        np.exp(mask2 + gb[b]).T).astype(ml_dtypes.bfloat16) for b in range(2)]
    x = np.asarray(x, dtype=np.float32)
    xnp = np.float32 if A_DT == F32R else ml_dtypes.bfloat16
    wonp = np.float32 if C_DT == F32R else ml_dtypes.bfloat16
    xT_b = [np.ascontiguousarray(x[b].T).astype(xnp) for b in range(2)]
    wq = np.asarray(wq, dtype=np.float32)
    wk = np.asarray(wk, dtype=np.float32)
    wv = np.asarray(wv, dtype=np.float32)
    wo = np.asarray(wo, dtype=np.float32)

    in_maps = []
    for c in range(N_CORES):
        b = c % 2
        kvp = c // 2
        kvg = (2 * kvp, 2 * kvp + 1)
        qcols, orows = [], []
        for r in range(NREP):
            for kv in kvg:
                h = kv * NREP + r
                qcols.extend(range(h * HD, (h + 1) * HD))
                orows.extend(range(h * HD, (h + 1) * HD))
        kcols = []
        for kv in kvg:
            kcols.extend(range(kv * HD, (kv + 1) * HD))
        wqkv = np.concatenate(
            [wq[:, qcols], wk[:, kcols], wv[:, kcols]], axis=1)
        in_maps.append({
            "xT": xT_b[b],
            "ecombT": ecombT_b[b],
            "wqkv": np.ascontiguousarray(wqkv).astype(xnp),
            "wo": np.ascontiguousarray(wo[orows, :]).astype(wonp),
        })
    return in_maps


def gather_outputs(results, S=2048):
    y = np.zeros((2, S, D), dtype=np.float32)
    for c in range(N_CORES):
        y[c % 2] += np.asarray(results[c]["y"], dtype=np.float32)
    return y


def detect_causal(mask, graph_bias, S=2048):
    """True if every score tile strictly above the block diagonal is fully
    masked (so the kernel may skip it): those tiles then contribute exactly 0
    probability, matching the reference."""
    if S % 128:
        return False
    m = np.asarray(mask, dtype=np.float32).reshape(S, S)
    nb = S // 128
    blockmax = m.reshape(nb, 128, nb, 128).max(axis=(1, 3))
    upper = np.triu(np.ones((nb, nb), dtype=bool), k=1)
    if not upper.any():
        return False
    if not bool((blockmax[upper] < -1e8).all()):
        return False
    return float(np.abs(np.asarray(graph_bias)).max()) < 1e6


_PROGRAM_CACHE = {}


def _get_program(S, causal, loop_n=1):
    key = (S, causal, loop_n)
    if key not in _PROGRAM_CACHE:
        _PROGRAM_CACHE[key] = build_program(S=S, causal=causal, loop_n=loop_n)
    return _PROGRAM_CACHE[key]


def kernel(x, mask, graph_bias, wq, wk, wv, wo, start_pos=0):
    import time as _time

    S = x.shape[1]
    causal = detect_causal(mask, graph_bias, S=S)
    nc = _get_program(S, causal)
    in_maps = shard_inputs(x, mask, graph_bias, wq, wk, wv, wo, S=S)
    # The backend occasionally wedges (NRT_EXEC_UNIT_UNRECOVERABLE) and
    # recovers after a short wait; retry rather than failing the run.
    last = None
    for attempt in range(3):
        try:
            res = run_bass_kernel_spmd(nc, in_maps, core_ids=list(range(N_CORES)))
            return gather_outputs(res.results, S=S)
        except Exception as e:  # noqa: BLE001
            last = e
            _time.sleep(20 * (attempt + 1))
    raise last
